# revision 34
# baseline (speedup 1.0000x reference)
"""Trainium2 Bass kernel for nn_AblationAttention (sliding-window causal
attention, W=256, with per-head RMSNorm on q/k).

Key math fact: the reference's "genetic fitness" block adds log(fitness)[b,h,q]
to scores — a constant along the softmax (k) axis — so softmax is invariant to
it and the block is a no-op for the output. We compute plain sliding-window
causal attention.

Sharding: 8 cores = batch (2) x head-group (4 groups of 4 heads).
Each core: full x for its batch (pre-transposed on host), column-sliced
wq/wk/wv, row-sliced wo. Host sums the 4 head-group partials per batch.

v4: fully interleaved pipeline. Phase A (projections+rmsnorm, PE-heavy),
phase B (attention, ACT-heavy) and phase C (out-projection) are emitted
interleaved per 128-row tile so engine loads overlap instead of serializing.

Phase B is k-major: scores are computed transposed (sT[k,q]) one key-tile at
a time against the <=3 query-tiles whose sliding window contains it, so exp
emits p^T directly in the layout the o-matmul wants — no per-iteration PE
transposes or p copies. The causal/window mask is applied by zeroing invalid
probabilities post-exp with Pool-engine affine_selects (the middle query-tile
of each key-tile pair is always fully valid; the outer two are complementary
triangles). A ones-column appended to each V block makes the o-matmul
accumulate the softmax denominator as column 64 of each head block, so
normalization is one per-partition broadcast multiply per query tile.
All o-matmuls of a query tile are emitted as one contiguous blob because
PSUM accumulation groups within one tile must not interleave.

rmsnorm's rsqrt is computed as exp(-0.5*ln(x)) so every activation used
(square/ln/exp/copy) lives in one ACT table — no table thrash when phases
interleave.
"""

import sys

sys.path.insert(0, "/opt/trn_rl_repo")

import numpy as np
import ml_dtypes

import concourse.bass as bass
import concourse.tile as tile
from concourse import bacc, mybir
from concourse import bass_utils
from concourse.masks import make_identity
from concourse.hw_specs import get_activation_tables

# Problem constants (hardcoded per harness contract)
B, T, E, H, W = 2, 2048, 1024, 16, 256
D = E // H  # 64
NCORES = 8
HG = 4  # head-groups
HPG = H // HG  # heads per core = 4
COLS = HPG * D  # 256
VC = D + 1  # v block width incl ones column = 65
EPS = float(np.finfo(np.float32).eps)
FP = mybir.dt.float32
BF = mybir.dt.bfloat16
F8 = mybir.dt.float8e4
NT = T // 128  # 16 tiles
AF = mybir.ActivationFunctionType
ALU = mybir.AluOpType
DR = mybir.MatmulPerfMode.DoubleRow
WS = 32.0  # host-side weight scale so fp8 weight entries sit near 1.0

_cache = {}


def _ap(t, extra_off, dims):
    """Custom AP on tile view t: partition dim kept, free dims replaced."""
    return bass.AP(tensor=t.tensor, offset=t.offset + extra_off, ap=[t.ap[0]] + dims)


def _build():
    nc = bacc.Bacc(
        "TRN2",
        target_bir_lowering=False,
        debug=False,
        enable_asserts=False,
        num_devices=NCORES,
    )
    xT = nc.dram_tensor("xT", [E, T], BF, kind="ExternalInput").ap()
    wqk = nc.dram_tensor("wqk", [E, 2 * COLS], BF, kind="ExternalInput").ap()
    wv = nc.dram_tensor("wv", [E, COLS], BF, kind="ExternalInput").ap()
    wo = nc.dram_tensor("wo", [COLS, E], BF, kind="ExternalInput").ap()
    qn2 = nc.dram_tensor("qn2", [128, 1], FP, kind="ExternalInput").ap()
    kn2 = nc.dram_tensor("kn2", [128, 1], FP, kind="ExternalInput").ap()
    out = nc.dram_tensor("out", [T, E], BF, kind="ExternalOutput").ap()

    with tile.TileContext(nc) as tc:
        with (
            tc.tile_pool(name="singles", bufs=1) as singles,
            tc.tile_pool(name="xin", bufs=4) as xin,
            tc.tile_pool(name="work", bufs=4) as work,
            tc.tile_pool(name="stats", bufs=16) as stats,
            tc.tile_pool(name="outst", bufs=4) as outst,
            tc.tile_pool(name="ps", bufs=1, space="PSUM") as ps,
        ):
            # one ACT table covers square/ln/exp/copy — load it explicitly so
            # the table-load pass doesn't greedily thrash between smaller sets
            set_id = list(get_activation_tables(nc.m.arch)).index(
                "natural_log_exp_and_others"
            )
            nc.scalar.add_instruction(
                mybir.InstLoadActFuncSet(
                    name=f"I-{nc.next_id()}", act_func_set_id=set_id,
                    engine=mybir.EngineType.Activation,
                )
            )

            # ---- resident tensors ----
            wqk_sb = singles.tile([128, 8, 2 * COLS], BF, tag="wqk")
            wv_sb = singles.tile([128, 8, COLS], BF, tag="wv")
            wo_sb = singles.tile([128, 2, E], BF, tag="wo")
            qn_sb = singles.tile([128, 1], FP, tag="qn2")
            kn_sb = singles.tile([128, 1], FP, tag="kn2")
            idb_sb = singles.tile([128, 128], BF, tag="identb")
            eps_sb = singles.tile([128, 1], FP, tag="eps")
            qT_sb = singles.tile([128, 2, T], BF, tag="qT")
            kT_sb = singles.tile([128, 2, T], BF, tag="kT")
            # v blocks with a ones column per head: [t, kt, 4*(64 v | 1)]
            v_sb = singles.tile([128, NT, HPG * VC], BF, tag="vsb")
            hoT_sb = [singles.tile([128, 2, 512], BF, tag=f"hoT{g}", name=f"hoT{g}") for g in range(4)]

            xT_r = xT.rearrange("(k p) t -> p k t", p=128)
            xmap = {}

            def load_x(tc_i):
                # x loads ride the ACT DMA queue so they never queue behind
                # the output stores on the sync queue
                xmap[tc_i] = xin.tile([128, 8, 512], BF, tag="xT", name="x_t")
                for sub in range(4):
                    t0 = tc_i * 512 + sub * 128
                    nc.scalar.dma_start(
                        out=xmap[tc_i][:, :, sub * 128 : (sub + 1) * 128],
                        in_=xT_r[:, :, t0 : t0 + 128],
                    )

            # x chunk 0 is issued first: the first projection matmuls need it
            # plus wqk/wv; wo is not read until the first out-projection rounds
            load_x(0)
            nc.sync.dma_start(out=wqk_sb, in_=wqk.rearrange("(k p) c -> p k c", p=128))
            nc.sync.dma_start(out=wv_sb, in_=wv.rearrange("(k p) c -> p k c", p=128))
            nc.sync.dma_start(out=qn_sb, in_=qn2)
            nc.sync.dma_start(out=kn_sb, in_=kn2)
            make_identity(nc, idb_sb)
            nc.vector.memset(eps_sb, EPS)
            # ones columns of v blocks: [128, NT, HPG] strided at offset 64
            nc.vector.memset(_ap(v_sb, D, [[HPG * VC, NT], [VC, HPG]]), 1.0)

            nrm_map = {}    # m -> (nrm_q, nrm_k)
            pT_map = {}     # (kt, h) -> pT tile
            onrm_map = {}   # qt -> (o_nrm hp0, o_nrm hp1)

            # ---- S2: projections + rmsnorm stats + normalize (PE/ACT/DVE) --
            def emit_a_mm(m):
                tc_i, ml = m // 4, m % 4
                x_t = xmap[tc_i]
                sl = slice(ml * 128, (ml + 1) * 128)
                qk_ps = ps.tile([128, 2 * COLS], FP, tag="qk_ps", bufs=1)
                vo = ps.tile([128, HPG * VC], FP, tag="vo", bufs=2)
                v_ps = vo[:, : COLS]
                for kc in range(8):
                    nc.tensor.matmul(qk_ps, x_t[:, kc, sl], wqk_sb[:, kc, :],
                                     start=(kc == 0), stop=(kc == 7))
                for kc in range(8):
                    nc.tensor.matmul(v_ps, x_t[:, kc, sl], wv_sb[:, kc, :],
                                     start=(kc == 0), stop=(kc == 7))
                # v: strided copy into the 65-wide head blocks (bf16 cast)
                vdst = _ap(v_sb[:, m, :], 0, [[VC, HPG], [1, D]])
                nc.vector.tensor_copy(vdst, v_ps.rearrange("p (h d) -> p h d", h=HPG))
                # rmsnorm stats for q and k together
                sq = work.tile([128, 2 * COLS], FP, tag="sq")
                nc.scalar.activation(sq, qk_ps, AF.Square)
                ssq8 = stats.tile([128, 2 * HPG], FP, tag="ssq8")
                nc.vector.tensor_reduce(
                    ssq8, sq.rearrange("p (g d) -> p g d", g=2 * HPG),
                    axis=mybir.AxisListType.X, op=ALU.add,
                )
                # rstd = exp(-0.5 * ln(ms + eps)) — keeps ACT in one table
                lns = stats.tile([128, 2 * HPG], FP, tag="lns")
                nc.scalar.activation(lns, ssq8, AF.Ln, bias=eps_sb, scale=1.0 / D)
                rstd8 = stats.tile([128, 2 * HPG], FP, tag="rstd8")
                nc.scalar.activation(rstd8, lns, AF.Exp, scale=-0.5)
                nrms = []
                for qki in range(2):
                    t_ps = qk_ps[:, qki * COLS : (qki + 1) * COLS]
                    nrm = work.tile([128, COLS], BF, tag="nrm", bufs=6)
                    rsl = rstd8[:, qki * HPG : (qki + 1) * HPG]
                    rstd_b = bass.AP(
                        tensor=rsl.tensor, offset=rsl.offset,
                        ap=[rsl.ap[0], [rsl.ap[1][0], HPG], [0, D]],
                    )
                    nc.vector.tensor_mul(
                        nrm.rearrange("p (g d) -> p g d", g=HPG),
                        t_ps.rearrange("p (g d) -> p g d", g=HPG),
                        rstd_b,
                    )
                    nrms.append(nrm)
                nrm_map[m] = nrms

            # ---- S4: transpose q/k tile m into qT/kT (PE + DVE/ACT) --------
            def emit_a_T(m):
                nrm_q, nrm_k = nrm_map.pop(m)
                t4 = ps.tile([128, 512], BF, tag="tps", bufs=2)
                for qki, nrm in ((0, nrm_q), (1, nrm_k)):
                    for hp in range(2):
                        nc.tensor.transpose(
                            t4[:, qki * 256 + hp * 128 : qki * 256 + (hp + 1) * 128],
                            nrm[:, hp * 128 : (hp + 1) * 128], idb_sb,
                        )
                for qki, (w_ap, dst) in enumerate(((qn_sb, qT_sb), (kn_sb, kT_sb))):
                    dview = dst[:, :, m * 128 : (m + 1) * 128]
                    t2v = t4[:, qki * 256 : (qki + 1) * 256].rearrange(
                        "p (a b) -> p a b", a=2
                    )
                    if qki == 0:
                        nc.vector.tensor_scalar_mul(dview, t2v, w_ap)
                    else:
                        nc.scalar.activation(dview, t2v, AF.Copy, scale=w_ap)

            # ---- S1: scores + exp + mask-zeroing (PE/ACT/Pool) -------------
            def emit_b(kt, heads):
                nq = min(3, NT - kt)
                c0 = kt * 128
                for hp, hi in heads:
                    if True:
                        h = hp * 2 + hi
                        po = slice(hi * 64, hi * 64 + 64)
                        s_ps = ps.tile([128, 384], FP, tag="s_ps", bufs=2, name="s_ps")
                        nc.tensor.matmul(
                            s_ps[:, : nq * 128],
                            kT_sb[po, hp, c0 : c0 + 128],
                            qT_sb[po, hp, c0 : c0 + nq * 128],
                            start=True, stop=True,
                        )
                        pT = work.tile([128, 384], BF, tag="pT", name="pT", bufs=20)
                        pT_map[(kt, h)] = pT
                        nc.scalar.activation(pT[:, : nq * 128], s_ps[:, : nq * 128], AF.Exp)
                        # j=0 block (qt==kt): keep where q_local >= k_local
                        nc.gpsimd.affine_select(
                            pT[:, 0:128], pT[:, 0:128],
                            pattern=[[1, 128]], compare_op=ALU.is_ge, fill=0.0,
                            base=0, channel_multiplier=-1,
                        )
                        # j=2 block (qt==kt+2): keep where k_local > q_local
                        if nq == 3:
                            nc.gpsimd.affine_select(
                                pT[:, 256:384], pT[:, 256:384],
                                pattern=[[-1, 128]], compare_op=ALU.is_ge, fill=0.0,
                                base=-1, channel_multiplier=1,
                            )

            # ---- S5: o-matmuls + normalize (PE + DVE) ----------------------
            def emit_o(qt):
                # all o-matmuls of a query tile in one contiguous blob:
                # accumulation groups in one PSUM tile must not interleave
                first = max(0, qt - 2)
                o_ps = ps.tile([128, HPG * VC], FP, tag="vo", bufs=2, name="o_ps")
                for h in range(HPG):
                    for kt2 in range(first, qt + 1):
                        j = qt - kt2
                        nc.tensor.matmul(
                            o_ps[:, h * VC : (h + 1) * VC],
                            pT_map[(kt2, h)][:, j * 128 : (j + 1) * 128],
                            v_sb[:, kt2, h * VC : (h + 1) * VC],
                            start=(kt2 == first), stop=(kt2 == qt),
                        )
                    if qt >= 2:
                        del pT_map[(qt - 2, h)]
                rec4 = stats.tile([128, HPG], FP, tag="rec4", name="rec4")
                nc.vector.reciprocal(rec4, _ap(o_ps, D, [[VC, HPG]]))
                onrms = []
                for hp in range(2):
                    o_nrm = work.tile([128, 128], BF, tag="o_nrm", name="o_nrm", bufs=6)
                    nc.vector.tensor_mul(
                        o_nrm.rearrange("p (a d) -> p a d", a=2),
                        _ap(o_ps, hp * 2 * VC, [[VC, 2], [1, D]]),
                        _ap(rec4, hp * 2, [[1, 2], [0, D]]),
                    )
                    onrms.append(o_nrm)
                onrm_map[qt] = onrms

            # ---- S6: transpose o into hoT (PE + DVE/ACT) -------------------
            def emit_oT(qt):
                onrms = onrm_map.pop(qt)
                g, r = qt // 4, qt % 4
                oT_ps = ps.tile([128, 512], BF, tag="tps", name="oT_ps", bufs=2)
                for hp in range(2):
                    nc.tensor.transpose(
                        oT_ps[:, hp * 128 : (hp + 1) * 128], onrms[hp], idb_sb
                    )
                    dst = hoT_sb[g][:, hp, r * 128 : (r + 1) * 128]
                    nc.vector.tensor_copy(dst, oT_ps[:, hp * 128 : (hp + 1) * 128])

            # ---- S3/S7: out-projection half (PE + DVE/ACT + DMA) -----------
            c_sb = {}

            def emit_c_half(m, nch):
                g, r = m // 4, m % 4
                rsl = slice(r * 128, (r + 1) * 128)
                if nch == 0:
                    c_sb[m] = outst.tile([128, E], BF, tag="o_sb", name="o_sb")
                o_sb = c_sb[m] if nch == 0 else c_sb.pop(m)
                nsl = slice(nch * 512, (nch + 1) * 512)
                c_ps = ps.tile([128, 512], FP, tag="c_ps", bufs=1)
                for c in range(2):
                    nc.tensor.matmul(
                        c_ps, hoT_sb[g][:, c, rsl], wo_sb[:, c, nsl],
                        start=(c == 0), stop=(c == 1),
                    )
                if nch == 0:
                    nc.scalar.copy(o_sb[:, nsl], c_ps)
                else:
                    nc.vector.tensor_copy(o_sb[:, nsl], c_ps)
                nc.sync.dma_start(
                    out=out[m * 128 : (m + 1) * 128, nsl], in_=o_sb[:, nsl]
                )

            # ---- pipelined rounds: each stage's deps are >=1 round old -----
            # scores are split around the projection matmuls so the s_ps
            # double-buffer rotation never makes PE wait on an in-flight exp;
            # the o-matmul blob sits at the very end of the round so its
            # youngest dependency (this key tile's last exp+selects) is a
            # full round old by the time PE reaches it
            for r in range(NT + 9):
                if 5 <= r <= NT + 4:
                    emit_b(r - 5, [(0, 0), (0, 1)])   # S1a
                if r < NT:
                    emit_a_mm(r)         # S2
                if r == 0:
                    # behind x-chunk 0 in the queue; first read at round 8
                    nc.sync.dma_start(
                        out=wo_sb, in_=wo.rearrange("(k p) e -> p k e", p=128)
                    )
                if 2 <= r <= 10 and (r + 2) % 4 == 0:
                    load_x((r + 2) // 4)  # prefetch x chunk 2 rounds early
                if 5 <= r <= NT + 4:
                    emit_b(r - 5, [(1, 0), (1, 1)])   # S1b
                if 8 <= r <= NT + 7:
                    emit_c_half(r - 8, 0)  # S3: first half of out tile r-8
                if 2 <= r <= NT + 1:
                    emit_a_T(r - 2)      # S4
                if 7 <= r <= NT + 6:
                    emit_oT(r - 7)       # S6
                if 8 <= r <= NT + 7:
                    emit_c_half(r - 8, 1)  # S7: second half of out tile r-8
                if 6 <= r <= NT + 5:
                    emit_o(r - 6)        # S5 (last: youngest deps, most margin)

    nc.compile()
    return nc


def _host_inputs(x, wq, wk, wv, wo, qn_w, kn_w):
    """Build the 8 per-core input maps."""
    qn2 = (np.tile(qn_w, 2) * 0.125).astype(np.float32)[:, None]
    kn2 = np.tile(kn_w, 2).astype(np.float32)[:, None]

    bf = ml_dtypes.bfloat16
    xT = np.ascontiguousarray(np.transpose(x, (0, 2, 1))).astype(bf)  # [B, E, T]
    in_maps = []
    for core in range(NCORES):
        b, g = divmod(core, HG)
        cs = slice(g * COLS, (g + 1) * COLS)
        wqk = np.concatenate([wq[:, cs], wk[:, cs]], axis=1).astype(bf)
        in_maps.append(
            {
                "xT": xT[b],
                "wqk": np.ascontiguousarray(wqk),
                "wv": np.ascontiguousarray(wv[:, cs]).astype(bf),
                "wo": np.ascontiguousarray(wo[cs, :]).astype(bf),
                "qn2": qn2,
                "kn2": kn2,
            }
        )
    return in_maps


def run(trace=False, **inputs):
    if "nc" not in _cache:
        _cache["nc"] = _build()
    nc = _cache["nc"]
    in_maps = _host_inputs(
        np.asarray(inputs["x"]), np.asarray(inputs["wq"]), np.asarray(inputs["wk"]),
        np.asarray(inputs["wv"]), np.asarray(inputs["wo"]),
        np.asarray(inputs["qn_w"]), np.asarray(inputs["kn_w"]),
    )
    res = bass_utils.run_bass_kernel_spmd(
        nc, in_maps, core_ids=list(range(NCORES)), trace=trace
    )
    bo = np.asarray(inputs["bo"], np.float32)
    outs = []
    for b in range(B):
        acc = np.zeros((T, E), np.float32)
        for g in range(HG):
            acc += np.asarray(res.results[b * HG + g]["out"], np.float32)
        outs.append(acc + bo[None, :])
    return np.stack(outs), res


def kernel(**inputs):
    out, _ = run(trace=False, **inputs)
    return out
